# revision 33
# baseline (speedup 1.0000x reference)
"""Local causal (sliding-window) attention kernel for Trainium2, SPMD over 8 cores.

Problem: states [4, 4096, 1024] f32; q/k/v = states @ W*.T + b*; each query t
attends keys t-8..t (window=8), softmax over valid positions, out = attn @ v.

Sharding: data-parallel, 8 shards = 4 batches x 2 sequence halves (2048 queries
each). The host supplies each shard's states pre-transposed to [H, 2056] with an
8-col halo (zero-padded at sequence start; masked out via the additive mask).

Score reformulation (saves one full GEMM): q.k = x_t^T A x_k + g.x_t + w.x_k
+ c0 with A = (Wq/sqrt(H))^T Wk precomputed on host. The device computes the
query-side projection Q' = A^T @ X (one GEMM) instead of both Q and K
projections; X itself serves as the score rhs, so the key side needs no halo
projection at all. The rank-1 term u[k] = w.x_k is a host GEMV shipped
partition-broadcast and added on DVE together with the band mask; per-query
terms/constants cancel in softmax, so no max-subtraction is needed (scores
are O(1), masked lanes get -30000 and underflow to exact zero in exp).

Scheduling notes (from perfetto traces): ~7us fixed program preamble before
the first DMA descriptor issues; each DIRECT2D issue costs ~630ns on its
queue; only sync/scalar have hardware DGE rings (8 rings, ~100GB/s each,
FIFO per ring), gpsimd DMAs take slow software rings. So: bulk tensors go
out as ~256KB per-chunk descriptors alternating between the sync and scalar
queues in consumption order, consts packed into one bf16 descriptor, and
warm-up matmuls on a memset tile keep the PE HAM-warm through the load
phase. V-GEMM groups are emitted before the Q' segment so the earliest
landing data is consumed first; output is DMA'd as bf16 halves (host
upcasts to f32).
"""

import numpy as np
import ml_dtypes

import concourse.bacc as bacc
import concourse.mybir as mybir
import concourse.tile as tile
from concourse.bass_utils import run_bass_kernel_spmd

B, T, H = 4, 4096, 1024
NCORES = 8
TC = T // 2            # queries per core
HALO = 8               # window size
TH = TC + HALO         # shard cols incl. halo
SPAN = 128 + HALO      # key span per 128-query tile
NT = TC // 128         # query tiles per core
HC = H // 128          # 128-row chunks of H
F32 = mybir.dt.float32
BF16 = mybir.dt.bfloat16
BF = ml_dtypes.bfloat16
AF = mybir.ActivationFunctionType
NWARM = 30             # HAM warm-up matmuls during the DMA load phase
CB_W = 128 + H + 2 * SPAN   # packed consts: [id | bv | m0 | mr]

_cache = {}


def _emit(nc, tc, aps, pools):
    (x_d, a_d, wv_d, cb_d, wt_d, vt_d, out_d) = aps
    consts, xw, acts, psP, psS, psT, psO, attn = pools

    cb = consts.tile([128, CB_W], BF16, tag="cb", name="cb")
    id_t = cb[:, 0:128]
    bv_t = cb[:, 128:128 + H]
    m0_t = cb[:, 128 + H:128 + H + SPAN]
    mr_t = cb[:, 128 + H + SPAN:CB_W]
    wt_t = consts.tile([128, HC], F32, tag="wt", name="wt_t")
    warm = consts.tile([128, 512], BF16, tag="warm", name="warm")

    xt = [xw.tile([128, TH], BF16, tag=f"x{c}", name=f"x{c}") for c in range(HC)]
    a_all = xw.tile([128, HC * H], BF16, tag="a_all", name="a_all")
    wv_all = xw.tile([128, HC * H], BF16, tag="wv_all", name="wv_all")
    at = [a_all[:, c * H:(c + 1) * H] for c in range(HC)]
    wvt = [wv_all[:, c * H:(c + 1) * H] for c in range(HC)]
    qt = [acts.tile([128, TC], BF16, tag=f"q{c}", name=f"q{c}")
          for c in range(HC)]
    vt = [acts.tile([128, H], BF16, tag=f"v{j}", name=f"v{j}")
          for j in range(NT)]
    vtail = acts.tile([128, H], BF16, tag="vtail", name="vtail")

    # --- DMA issue plan: sync/scalar HW queues only, ~256KB descriptors in
    # consumption order, alternating queues so consecutive tensors land on
    # different DMA rings.
    nc.gpsimd.memset(warm[:], 1.0)
    nc.scalar.dma_start(cb[:], cb_d[:])
    nc.scalar.dma_start(wt_t[:], wt_d[:])
    for c in range(HC):       # wv first: V group 0 consumes it first
        (nc.sync if c % 2 else nc.scalar).dma_start(
            wv_all[:, c * H:(c + 1) * H], wv_d[:, c * H:(c + 1) * H])
    for c in range(HC):       # x cols 0:264 (V tiles 0-1 + attn 0-1)
        (nc.scalar if c % 2 else nc.sync).dma_start(
            xt[c][:, 0:264], x_d[c * 128:(c + 1) * 128, 0:264])
    for c in range(HC):       # x cols 264:520 (V tiles 2-3, Q' seg 0)
        (nc.scalar if c % 2 else nc.sync).dma_start(
            xt[c][:, 264:520], x_d[c * 128:(c + 1) * 128, 264:520])
    for c in range(HC):       # A chunks
        (nc.sync if c % 2 else nc.scalar).dma_start(
            a_all[:, c * H:(c + 1) * H], a_d[:, c * H:(c + 1) * H])
    nc.scalar.dma_start(vtail[:], vt_d[:])
    for seg in range(3):      # remaining x column segments: same HW rings,
        lo = 520 + seg * 512  # FIFO order naturally deprioritizes them
        for c in range(HC):
            nc.sync.dma_start(
                xt[c][:, lo:lo + 512], x_d[c * 128:(c + 1) * 128, lo:lo + 512])

    # Dedicated P^T staging buffers (3-deep rotation). Rows 8:128 of the
    # right half are zeroed once and never rewritten: the PV tail matmul can
    # then use a full 128-row lhsT (zero weights kill the garbage), which
    # keeps LDWEIGHTS pipelined (8-row weight loads stall the PE).
    ptabs = [consts.tile([128, 256], BF16, tag=f"ptab{k}", name=f"ptab{k}")
             for k in range(3)]
    for k in range(3):
        nc.gpsimd.memset(ptabs[k][:, 128:256], 0.0)

    # --- HAM warm-up: junk matmuls on the memset tile fill the PE during
    # the load phase so real matmuls start at full clock.
    for w in range(NWARM):
        ps = psP.tile([128, 512], F32, tag="ps", name="pswarm")
        nc.tensor.matmul(ps[:], warm[:, 0:128], warm[:], start=True, stop=True)

    def emit_q(t4):
        off = t4 * 512
        for hc in range(HC):
            ps = psP.tile([128, 512], F32, tag="ps", name="psq")
            for c in range(HC):
                nc.tensor.matmul(
                    ps[:], at[c][:, hc * 128:(hc + 1) * 128],
                    xt[c][:, HALO + off: HALO + off + 512],
                    start=(c == 0), stop=(c == HC - 1))
            # alternate copy engine to balance Scalar/Vector queues; the
            # per-partition wt bias folds the rank-1 u[k]=wt.x_k score term
            # into the Q' operand: (q'+wt).x_k = q'.x_k + u_k
            if hc % 2 == 0:
                nc.scalar.add(qt[hc][:, off: off + 512], ps[:],
                              wt_t[:, hc:hc + 1])
            else:
                nc.vector.tensor_scalar_add(qt[hc][:, off: off + 512], ps[:],
                                            wt_t[:, hc:hc + 1])

    def emit_v(j):
        # c-major over both hh psums: consecutive matmuls share the same
        # stationary operand (weight-load reuse)
        pss = [psP.tile([128, 512], F32, tag="ps", name=f"psv{hh}")
               for hh in range(2)]
        for c in range(HC):
            for hh in range(2):
                nc.tensor.matmul(
                    pss[hh][:], xt[c][:, j * 128: (j + 1) * 128],
                    wvt[c][:, hh * 512:(hh + 1) * 512],
                    start=(c == 0), stop=(c == HC - 1))
        for hh in range(2):
            nc.vector.tensor_add(
                vt[j][:, hh * 512:(hh + 1) * 512], pss[hh][:],
                bv_t[:, hh * 512:(hh + 1) * 512])

    def emit_attn(j):
        s_ps = psS.tile([128, SPAN], F32, tag="s", name="s_ps")
        for c in range(HC):
            nc.tensor.matmul(
                s_ps[:], qt[c][:, j * 128: (j + 1) * 128],
                xt[c][:, j * 128: j * 128 + SPAN],
                start=(c == 0), stop=(c == HC - 1))
        s_sb = attn.tile([128, SPAN], F32, tag="ssb", name="s_sb")
        nc.vector.tensor_add(s_sb[:], s_ps[:],
                             (m0_t if j == 0 else mr_t)[:])
        p_bf = attn.tile([128, SPAN], BF16, tag="p", name="p_bf")
        rowsum = attn.tile([128, 1], F32, tag="rs", name="rowsum")
        nc.scalar.activation(p_bf[:], s_sb[:], AF.Exp,
                             bias=0.0, scale=1.0,
                             accum_out=rowsum[:])
        rinv = attn.tile([128, 1], F32, tag="ri", name="rinv")
        nc.vector.reciprocal(rinv[:], rowsum[:])

        pt_ps = psT.tile([128, 256], BF16, tag="pt", name="pt_ps")
        nc.tensor.transpose(pt_ps[:, 0:128], p_bf[:, 0:128], id_t[:])
        nc.tensor.transpose(pt_ps[:HALO, 128:256], p_bf[:, 128:SPAN], id_t[:])
        ptab = ptabs[j % 3]
        nc.scalar.copy(ptab[:, 0:128], pt_ps[:, 0:128])
        nc.vector.tensor_copy(ptab[:HALO, 128:256], pt_ps[:HALO, 128:256])

        vnext = vtail if j == NT - 1 else vt[j + 1]
        out_sb = attn.tile([128, H], BF16, tag="osb", name="out_sb")
        # both hh chains open at once, grouped by stationary operand so the
        # pta/ptb weight loads are each reused across the two psums
        o_pss = [psO.tile([128, 512], F32, tag="o", name=f"o_ps{hh}")
                 for hh in range(2)]
        for hh in range(2):
            nc.tensor.matmul(o_pss[hh][:], ptab[:, 0:128],
                             vt[j][:, hh * 512:(hh + 1) * 512],
                             start=True, stop=False)
        for hh in range(2):
            nc.tensor.matmul(o_pss[hh][:], ptab[:, 128:256],
                             vnext[:, hh * 512:(hh + 1) * 512],
                             start=False, stop=True)
        for hh in range(2):
            if hh == 0:
                nc.scalar.activation(
                    out_sb[:, hh * 512:(hh + 1) * 512], o_pss[hh][:],
                    AF.Copy, bias=0.0, scale=rinv[:])
            else:
                nc.vector.tensor_scalar_mul(
                    out_sb[:, hh * 512:(hh + 1) * 512], o_pss[hh][:], rinv[:])
            nc.sync.dma_start(
                out_d[j * 128:(j + 1) * 128, hh * 512:(hh + 1) * 512],
                out_sb[:, hh * 512:(hh + 1) * 512])

    # Interleave: V group first (its data lands first), then Q' segment,
    # then attention tiles whose queries fit the Q' columns produced so far
    # (attn j needs Q' cols < 512*(t4+1) and vt[j+1]). In the final block the
    # last V chains are interleaved after Q'3 so they hide part of the
    # closing attention burst.
    for t4 in range(TC // 512 - 1):
        for j in range(4 * t4, 4 * t4 + 4):
            emit_v(j)
        if t4 > 0:
            emit_attn(4 * t4 - 1)
        emit_q(t4)
        for j in range(4 * t4, 4 * t4 + 3):
            emit_attn(j)
    emit_v(12)
    emit_v(13)
    emit_attn(11)
    emit_q(3)
    emit_v(14)
    emit_attn(12)
    emit_v(15)
    emit_attn(13)
    emit_attn(14)
    emit_attn(15)


def _build(loop_reps=None, trace_sim=False):
    key = ("nc", loop_reps, trace_sim)
    if key in _cache:
        return _cache[key]
    nc = bacc.Bacc("TRN2", target_bir_lowering=False, debug=False,
                   num_devices=NCORES)

    aps = (
        nc.dram_tensor("x", [H, TH], BF16, kind="ExternalInput").ap(),
        nc.dram_tensor("a", [128, HC * H], BF16, kind="ExternalInput").ap(),
        nc.dram_tensor("wv", [128, HC * H], BF16, kind="ExternalInput").ap(),
        nc.dram_tensor("cb", [128, CB_W], BF16, kind="ExternalInput").ap(),
        nc.dram_tensor("wt", [128, HC], F32, kind="ExternalInput").ap(),
        nc.dram_tensor("vtail", [128, H], BF16, kind="ExternalInput").ap(),
        nc.dram_tensor("out", [TC, H], BF16, kind="ExternalOutput").ap(),
    )

    with tile.TileContext(nc, trace_sim=trace_sim) as tc:
        with (
            tc.tile_pool(name="consts", bufs=1) as consts,
            tc.tile_pool(name="xw", bufs=1) as xw,
            tc.tile_pool(name="acts", bufs=1) as acts,
            tc.tile_pool(name="psP", bufs=3, space="PSUM") as psP,
            tc.tile_pool(name="psS", bufs=2, space="PSUM") as psS,
            tc.tile_pool(name="psT", bufs=1, space="PSUM") as psT,
            tc.tile_pool(name="psO", bufs=2, space="PSUM") as psO,
            tc.tile_pool(name="attn", bufs=4) as attn,
        ):
            pools = (consts, xw, acts, psP, psS, psT, psO, attn)
            if loop_reps:
                with tc.For_i(0, loop_reps, 1):
                    _emit(nc, tc, aps, pools)
            else:
                _emit(nc, tc, aps, pools)

    nc.compile()
    _cache[key] = nc
    return nc


def _host_inputs(states, Wq, bq, Wk, bk, Wv, bv):
    """Shared (per-run) host-side tensor prep."""
    scale = 1.0 / np.sqrt(H)
    Wq = np.asarray(Wq, np.float32)
    Wk = np.asarray(Wk, np.float32)
    Wv = np.asarray(Wv, np.float32)
    bq = np.asarray(bq, np.float32)
    bv = np.asarray(bv, np.float32)
    Wqs = Wq * scale
    # S = x_q^T A x_k with A = Wqs^T Wk; Q' = A^T X needs lhsT = A chunks
    a_h = np.ascontiguousarray(Wqs.T @ Wk).astype(BF)
    wt_h = Wk.T @ (bq * scale)   # u[k] = wt . x_k
    wv_h = np.ascontiguousarray(Wv.T).astype(BF)
    # chunk-packed [128, 8*H] device layouts: block c = rows c*128..c*128+127
    a_p = np.ascontiguousarray(
        a_h.reshape(HC, 128, H).transpose(1, 0, 2).reshape(128, HC * H))
    wv_p = np.ascontiguousarray(
        wv_h.reshape(HC, 128, H).transpose(1, 0, 2).reshape(128, HC * H))
    m = np.arange(128)[:, None]
    n = np.arange(SPAN)[None, :]
    band = (n >= m) & (n <= m + HALO)
    mr_h = np.where(band, 0.0, -30000.0).astype(BF)
    m0_h = np.where(band & (n >= HALO), 0.0, -30000.0).astype(BF)
    id_h = np.eye(128).astype(BF)
    bv_b = np.broadcast_to(bv.astype(BF), (128, H))
    wt_c = np.ascontiguousarray(
        wt_h.astype(np.float32).reshape(HC, 128).T)  # [128, HC]
    cb0 = np.ascontiguousarray(np.concatenate(
        [id_h, bv_b, m0_h, mr_h], axis=1))
    cbr = np.ascontiguousarray(np.concatenate(
        [id_h, bv_b, mr_h, mr_h], axis=1))
    return a_p, wt_c, wv_h, wv_p, cb0, cbr, bv


def _shard_maps(states, hosts):
    a_p, wt_c, wv_h, wv_p, cb0, cbr, bv = hosts
    wv_f = wv_h.astype(np.float32)
    in_maps = []
    for i in range(NCORES):
        b, hf = i // 2, i % 2
        xs = np.zeros((TH, H), np.float32)
        if hf == 0:
            xs[HALO:] = states[b, 0:TC]
        else:
            xs[:] = states[b, TC - HALO: 2 * TC]
        x_h = np.ascontiguousarray(xs.T).astype(BF)   # [H, TH]
        x_f = x_h.astype(np.float32)
        vtail_h = np.zeros((128, H), BF)               # rows 8: stay zero
        vtail_h[:HALO] = (x_f[:, TC:].T @ wv_f + bv).astype(BF)
        in_maps.append({
            "x": x_h, "a": a_p, "wv": wv_p,
            "cb": (cb0 if hf == 0 else cbr), "wt": wt_c,
            "vtail": vtail_h,
        })
    return in_maps


def kernel(states, Wq, bq, Wk, bk, Wv, bv, window):
    assert int(window) == HALO
    states = np.asarray(states, np.float32)
    nc = _build()
    hosts = _host_inputs(states, Wq, bq, Wk, bk, Wv, bv)
    in_maps = _shard_maps(states, hosts)
    res = run_bass_kernel_spmd(nc, in_maps, list(range(NCORES)))
    out = np.empty((B, T, H), np.float32)
    for i in range(NCORES):
        b, hf = i // 2, i % 2
        out[b, hf * TC:(hf + 1) * TC] = res.results[i]["out"].astype(np.float32)
    return out
